# revision 1
# baseline (speedup 1.0000x reference)
"""MoE layer (24 experts, top-2) on 8 Trainium2 NeuronCores, expert-parallel.

Strategy: host computes the gate routing (replicated-gate equivalent), then
dispatches each expert's tokens to the core that owns that expert (3 experts
per core, count-balanced).  Each core runs one Bass/Tile program computing, for
each of its 3 expert slots:   Y^T = w2^T-contract(gelu(w1-contract(x^T)+b1))
with tokens on the matmul free dim (so per-expert token counts need no 128
padding), then applies +b2 and the per-token combine weight on-device.  Host
scatter-adds the per-expert outputs back (the "combine" all-to-all).

Matmuls run in float32r (reduced-precision fp32 PE mode): same per-row speed
as bf16 for free dims >= 256, ~1.5e-4 relative error.
"""

import sys

for _p in ("/opt/trn_rl_repo", "/root/.axon_site/_ro/trn_rl_repo"):
    if _p not in sys.path:
        sys.path.append(_p)

import numpy as np

import concourse.bass as bass  # noqa: F401  (AP helpers)
import concourse.tile as tile
from concourse import bacc, mybir
from concourse.bass_utils import run_bass_kernel_spmd

B, S, D, FF, E, TOPK = 4, 1024, 1024, 4096, 24, 2
T = B * S
P = 128
KT1 = D // P     # 8  k-subtiles for MM1
MT1 = FF // P    # 32 f-tiles (MM1 output partitions)
KT2 = FF // P    # 32 k-subtiles for MM2
MT2 = D // P     # 8  d-tiles (MM2 output partitions)
N_CORES = 8
SLOTS = E // N_CORES  # 3 experts per core

F32R = mybir.dt.float32r
F32 = mybir.dt.float32

_program_cache: dict = {}


def _build_program(caps):
    """One SPMD program: 3 expert slots with token capacities caps[j]."""
    nc = bacc.Bacc("TRN2", target_bir_lowering=False, debug=False)

    w1t = nc.dram_tensor("w1t", (SLOTS, MT1, P, KT1, P), F32R, kind="ExternalInput")
    w2t = nc.dram_tensor("w2t", (SLOTS, MT2, P, KT2, P), F32R, kind="ExternalInput")
    b1t = nc.dram_tensor("b1t", (SLOTS, P, MT1), F32, kind="ExternalInput")
    b2t = nc.dram_tensor("b2t", (SLOTS, P, MT2), F32, kind="ExternalInput")
    xgs = [nc.dram_tensor(f"xg{j}", (P, KT1, caps[j]), F32R, kind="ExternalInput")
           for j in range(SLOTS)]
    gws = [nc.dram_tensor(f"gw{j}", (P, caps[j]), F32, kind="ExternalInput")
           for j in range(SLOTS)]
    ygs = [nc.dram_tensor(f"yg{j}", (MT2, P, caps[j]), F32, kind="ExternalOutput")
           for j in range(SLOTS)]

    with tile.TileContext(nc) as tc:
        with tc.tile_pool(name="xg", bufs=2) as xg_pool, \
             tc.tile_pool(name="gw", bufs=2) as gw_pool, \
             tc.tile_pool(name="bias", bufs=2) as bias_pool, \
             tc.tile_pool(name="w1", bufs=4) as w1_pool, \
             tc.tile_pool(name="w2", bufs=3) as w2_pool, \
             tc.tile_pool(name="h", bufs=MT1) as h_pool, \
             tc.tile_pool(name="epi", bufs=4) as epi_pool, \
             tc.tile_pool(name="psa", bufs=4, space="PSUM") as psa, \
             tc.tile_pool(name="psb", bufs=4, space="PSUM") as psb:
            for j in range(SLOTS):
                C = caps[j]
                xg_sb = xg_pool.tile([P, KT1, C], F32R, tag="xg")
                nc.sync.dma_start(xg_sb[:], xgs[j].ap()[:])
                gw_sb = gw_pool.tile([P, C], F32, tag="gw")
                nc.sync.dma_start(gw_sb[:], gws[j].ap()[:])
                b1_sb = bias_pool.tile([P, MT1], F32, tag="b1")
                nc.sync.dma_start(b1_sb[:], b1t.ap()[j])
                b2_sb = bias_pool.tile([P, MT2], F32, tag="b2")
                nc.sync.dma_start(b2_sb[:], b2t.ap()[j])

                # Phase A: H^T[f, t] = gelu(w1^T x^T + b1), f-tile at a time.
                h_tiles = []
                for m in range(MT1):
                    w1_sb = w1_pool.tile([P, KT1, P], F32R, tag="w1")
                    nc.sync.dma_start(w1_sb[:], w1t.ap()[j, m])
                    ph = psa.tile([P, C], F32, tag="psa")
                    for k in range(KT1):
                        nc.tensor.matmul(ph[:], w1_sb[:, k, :], xg_sb[:, k, :],
                                         start=(k == 0), stop=(k == KT1 - 1))
                    h_sb = h_pool.tile([P, C], F32R, tag="h")
                    nc.scalar.activation(h_sb[:], ph[:],
                                         mybir.ActivationFunctionType.Gelu,
                                         bias=b1_sb[:, m:m + 1])
                    h_tiles.append(h_sb)

                # Phase B: Y^T[d, t] = w2^T H^T; epilogue (+b2) * gate.
                for mo in range(MT2):
                    w2_sb = w2_pool.tile([P, KT2, P], F32R, tag="w2")
                    nc.sync.dma_start(w2_sb[:], w2t.ap()[j, mo])
                    py = psb.tile([P, C], F32, tag="psb")
                    for k in range(KT2):
                        nc.tensor.matmul(py[:], w2_sb[:, k, :], h_tiles[k][:],
                                         start=(k == 0), stop=(k == KT2 - 1))
                    yb = epi_pool.tile([P, C], F32, tag="yb")
                    nc.scalar.activation(yb[:], py[:],
                                         mybir.ActivationFunctionType.Identity,
                                         bias=b2_sb[:, mo:mo + 1])
                    yo = epi_pool.tile([P, C], F32, tag="yo")
                    nc.vector.tensor_mul(yo[:], yb[:], gw_sb[:])
                    nc.sync.dma_start(ygs[j].ap()[mo], yo[:])
    nc.compile()
    return nc


def _route(x2d, gate_w, gate_b):
    """fp32 gate scores -> top-2 -> softmax combine weights."""
    scores = x2d @ gate_w + gate_b                       # [T, E] fp32
    topi = np.argsort(-scores, axis=1, kind="stable")[:, :TOPK]   # [T, 2]
    topv = np.take_along_axis(scores, topi, axis=1)
    m = topv.max(axis=1, keepdims=True)
    g = np.exp(topv - m)
    g = g / g.sum(axis=1, keepdims=True)                 # [T, 2] fp32
    return topi, g.astype(np.float32)


def kernel(x, gate_w, gate_b, w1, b1, w2, b2):
    x = np.ascontiguousarray(np.asarray(x, dtype=np.float32))
    gate_w = np.asarray(gate_w, dtype=np.float32)
    gate_b = np.asarray(gate_b, dtype=np.float32)
    w1 = np.asarray(w1, dtype=np.float32)
    b1 = np.asarray(b1, dtype=np.float32)
    w2 = np.asarray(w2, dtype=np.float32)
    b2 = np.asarray(b2, dtype=np.float32)

    x2d = x.reshape(T, D)
    topi, gates = _route(x2d, gate_w, gate_b)

    # token lists per expert
    idx_e = [np.nonzero(topi == e)[0] for e in range(E)]  # token ids, sorted
    gv_e = []
    for e in range(E):
        rows = topi == e                                   # [T, 2] bool
        sel = rows.any(axis=1)
        gv_e.append(gates[rows.any(axis=1)][...] if False else
                    gates[sel, :][rows[sel, :]].astype(np.float32))
    counts = np.array([len(i) for i in idx_e])

    # balance experts over (core, slot): sort by count desc, slot j takes
    # ranks [8j, 8j+8); capacity per slot = max count in the slot, even.
    order = np.argsort(-counts, kind="stable")
    slot_expert = np.empty((N_CORES, SLOTS), dtype=int)
    caps = []
    for j in range(SLOTS):
        ranks = order[j * N_CORES:(j + 1) * N_CORES]
        slot_expert[:, j] = ranks
        cmax = int(counts[ranks].max())
        caps.append(cmax + (cmax & 1))
    caps = tuple(caps)

    key = caps
    if key not in _program_cache:
        _program_cache[key] = _build_program(caps)
    nc = _program_cache[key]

    xT = np.ascontiguousarray(x2d.T)                       # [D, T]
    in_maps = []
    for c in range(N_CORES):
        m = {}
        w1c = np.empty((SLOTS, MT1, P, KT1, P), np.float32)
        w2c = np.empty((SLOTS, MT2, P, KT2, P), np.float32)
        b1c = np.empty((SLOTS, P, MT1), np.float32)
        b2c = np.empty((SLOTS, P, MT2), np.float32)
        for j in range(SLOTS):
            e = int(slot_expert[c, j])
            C = caps[j]
            n = int(counts[e])
            # x gathered, transposed, tiled: [P, KT1, C]
            xg = np.zeros((P, KT1, C), np.float32)
            xg[:, :, :n] = xT[:, idx_e[e]].reshape(KT1, P, n).transpose(1, 0, 2)
            m[f"xg{j}"] = xg
            gw = np.zeros((C,), np.float32)
            gw[:n] = gv_e[e]
            m[f"gw{j}"] = np.broadcast_to(gw, (P, C)).copy()
            w1c[j] = w1[e].reshape(KT1, P, MT1, P).transpose(2, 1, 0, 3)
            w2c[j] = w2[e].reshape(KT2, P, MT2, P).transpose(2, 1, 0, 3)
            b1c[j] = b1[e].reshape(MT1, P).T
            b2c[j] = b2[e].reshape(MT2, P).T
        m["w1t"] = w1c
        m["w2t"] = w2c
        m["b1t"] = b1c
        m["b2t"] = b2c
        in_maps.append(m)

    res = run_bass_kernel_spmd(nc, in_maps, core_ids=list(range(N_CORES)))

    out = np.zeros((T, D), np.float32)
    for c in range(N_CORES):
        for j in range(SLOTS):
            e = int(slot_expert[c, j])
            n = int(counts[e])
            yg = res.results[c][f"yg{j}"].reshape(D, caps[j])  # [D, C]
            out[idx_e[e], :] += yg[:, :n].T
    return out.reshape(B, S, D)
